# revision 2
# baseline (speedup 1.0000x reference)
"""Trainium2 Bass kernel for MinibatchDiscrimination2d (v2, fp8 + PE tiling).

Full computation:
  x (32,128,64,64) --conv s4--> x_r (32,3,16,16)
  M = x_r @ T  -> (32, 8192, 16)
  dist[b1,b2,d] = sum_f |M[b1,d,f]-M[b2,d,f]|
  out[b,d] = sum_b2 exp(-dist) - 1 -> (32,32,16,16)
  out_a = deconv s4 (32,32,64,64); return concat([x, out_a], ch)

Sharding over 8 cores (core c owns t-rows 2c, 2c+1 of the 16x16 spatial
grid -> 1024 of the 8192 d values, all 32 samples; conv data-parallel
over b with an AllGather of the tiny x_r).

Per-core d index: d_local = (r, j, ch) with r in 2, j in 16, ch in 32.
Grouping: g = r*4 + j//4 (8 groups of 128), ncn = j%4, s32 = ch.
s_local within g = ncn*32 + ch.

Numerics: T is scaled x1024 and sent as fp8 e4m3 (TRN variant, max 240);
x is scaled x32 to fp8 with conv weights pre-divided by 32. M is kept as
1024*M in bf16; the exp activation uses scale = -1/1024. Host-emulated
global rel err ~5e-4 (gate 2e-2).

Engine structure per g:
  M: col-tiled x4 matmuls (tile_position=(0,32j)), T streamed fp8,
     psum M4 (128=(ncn,b), 512) accumulated over 6 din chunks.
  D: row-tiled x4 matmuls (tile_position=(32*ncn,0)), sgn weights,
     each reading Mb4[32ncn:32ncn+32, :]; two (128,1024) psum halves
     per pair-chunk pc.
  dist: vector fused abs-reduce over f=16 (psum f32, 1x).
  E=exp(-dist/1024) on scalar; acc += E^T @ inc on PE.
Deconv: 8 matmuls into psum laid out as the final (u*32+oc, b*64+w)
rows, DMA'd straight from psum to dram f32; host reassembles.
"""

import numpy as np
import ml_dtypes

N_CORES = 8
B, IN_FLT, N = 32, 128, 64
K = 4
T_SP = 16
OC = 32
F = 16
D_IN = 768
BC = B // N_CORES          # 4 samples per core (conv data-parallel)
DSH = 1024                 # d per core
NG = 8                     # dgroups of 128 s-values
KCH = 6                    # 128-row din chunks
TS = 1024.0                # T scale into fp8
XS = 32.0                  # x scale into fp8

_CACHE = {}


def _build_nc():
    import concourse.bacc as bacc
    import concourse.mybir as mybir
    import concourse.tile as tile

    f32 = mybir.dt.float32
    bf16 = mybir.dt.bfloat16
    fp8 = mybir.dt.float8e4
    AFT = mybir.ActivationFunctionType
    ALU = mybir.AluOpType

    nc = bacc.Bacc("TRN2", target_bir_lowering=False, debug=False,
                   num_devices=N_CORES)

    xc8 = nc.dram_tensor("xc8", [IN_FLT, BC * N * N], fp8, kind="ExternalInput")
    tsh = nc.dram_tensor("tsh", [NG, 128, KCH * 2048], fp8, kind="ExternalInput")
    wc = nc.dram_tensor("wc", [IN_FLT, 48], bf16, kind="ExternalInput")
    wd4 = nc.dram_tensor("wd4", [128, 2048], bf16, kind="ExternalInput")
    sgn4 = nc.dram_tensor("sgn4", [128, 512], bf16, kind="ExternalInput")
    inc = nc.dram_tensor("inc", [128, 128], bf16, kind="ExternalInput")
    eye = nc.dram_tensor("eye", [B, B], bf16, kind="ExternalInput")
    zrow = nc.dram_tensor("zrow", [1, 128], bf16, kind="ExternalInput")
    y = nc.dram_tensor("y", [2, 128, 2048], bf16, kind="ExternalOutput")

    with tile.TileContext(nc) as tc:
        with tc.tile_pool(name="const", bufs=1) as constp, \
             tc.tile_pool(name="dram", bufs=1, space="DRAM") as dram, \
             tc.tile_pool(name="xb", bufs=1) as xbp, \
             tc.tile_pool(name="Tp", bufs=2) as Tp, \
             tc.tile_pool(name="work", bufs=2) as wp, \
             tc.tile_pool(name="persist", bufs=1) as pp, \
             tc.tile_pool(name="ps_cv", bufs=1, space="PSUM") as ps_cv, \
             tc.tile_pool(name="ps_m", bufs=2, space="PSUM") as ps_m, \
             tc.tile_pool(name="ps_d", bufs=2, space="PSUM") as ps_d, \
             tc.tile_pool(name="ps_a", bufs=1, space="PSUM") as ps_a:

            wc_sb = constp.tile([IN_FLT, 48], bf16)
            nc.scalar.dma_start(wc_sb[:], wc[:])
            wd_sb = constp.tile([128, 2048], bf16)
            nc.scalar.dma_start(wd_sb[:], wd4[:])
            sgn_sb = constp.tile([128, 512], bf16)
            nc.scalar.dma_start(sgn_sb[:], sgn4[:])
            inc_sb = constp.tile([128, 128], bf16)
            nc.scalar.dma_start(inc_sb[:], inc[:])
            eye_sb = constp.tile([B, B], bf16)
            nc.scalar.dma_start(eye_sb[:], eye[:])
            zrow_sb = constp.tile([1, 128], bf16)
            nc.scalar.dma_start(zrow_sb[:], zrow[:])

            # ---- x shard load (fp8, split per b-pair) + T prefetch
            x8 = xbp.tile([IN_FLT, BC * N * N], fp8, tag="x8")
            for half in range(2):
                nc.sync.dma_start(x8[:, half * 8192:(half + 1) * 8192],
                                  xc8[:, half * 8192:(half + 1) * 8192])

            Ts = []
            for g in range(NG):
                Tt = Tp.tile([128, KCH * 2048], fp8, tag="T")
                nc.sync.dma_start(Tt[:], tsh[g, :, :])
                Ts.append(Tt)

            # ---- Stage A: conv.  x8 cols = (b(4), rs(16), ij(256))
            xrl = pp.tile([3, BC * 256], bf16)       # cols = (b, i, j)
            x8_rs = x8[:].rearrange(
                "c (b rs ij) -> c b rs ij", b=BC, rs=16)
            for half in range(2):
                psc_t = ps_cv.tile([128, 512], f32, tag="cv")
                psc = psc_t[:3, :]
                for idx in range(16):
                    nc.tensor.matmul(
                        psc[:].rearrange("p (b ij) -> p b ij", b=2),
                        wc_sb[:, idx * 3:idx * 3 + 3],
                        x8_rs[:, half * 2:(half + 1) * 2, idx],
                        start=(idx == 0), stop=(idx == 15))
                nc.scalar.copy(xrl[:, half * 512:(half + 1) * 512], psc[:])

            ag_in = dram.tile([BC, D_IN], bf16)
            ag_out = dram.tile([B, D_IN], bf16)
            nc.gpsimd.dma_start(
                ag_in[:].rearrange("b (c ij) -> c b ij", c=3),
                xrl[:].rearrange("c (b ij) -> c b ij", b=BC))
            nc.gpsimd.collective_compute(
                "AllGather", ALU.bypass,
                replica_groups=[list(range(N_CORES))],
                ins=[ag_in.opt()], outs=[ag_out.opt()])

            # ---- Stage B: x_r^T via eye-matmul (lhsT = xr slice, rhs = eye)
            xr_all = pp.tile([B, D_IN], bf16)
            nc.gpsimd.dma_start(xr_all[:], ag_out[:])
            xrT = pp.tile([128, KCH * B], bf16)
            for k in range(KCH):
                pst_t = ps_cv.tile([128, 512], f32, tag="cv")
                pst = pst_t[:, :B]
                nc.tensor.matmul(pst[:], xr_all[:, k * 128:(k + 1) * 128],
                                 eye_sb[:], start=True, stop=True)
                nc.scalar.copy(xrT[:, k * B:(k + 1) * B], pst[:])

            acc = pp.tile([128, NG * B], bf16)       # (s_local, (g, b))

            # ---- Stages C/D/E fused per dgroup g
            def emit_M(g):
                Tt = Ts[g]
                M4 = ps_m.tile([128, 512], f32, tag="M4")
                for k in range(KCH):
                    for j in range(4):
                        nc.tensor.matmul(
                            M4[32 * j:32 * (j + 1), :],
                            xrT[:, k * B:(k + 1) * B],
                            Tt[:, k * 2048 + j * 512:k * 2048 + (j + 1) * 512],
                            start=(k == 0), stop=(k == KCH - 1),
                            tile_position=(0, 32 * j))
                return M4

            def emit_accg(g, Eg):
                accg_t = ps_a.tile([128, 512], f32, tag="accg")
                accg = accg_t[:, :B]
                for pc in range(4):
                    nc.tensor.matmul(
                        accg[:], Eg[:, pc * 128:(pc + 1) * 128],
                        inc_sb[:, pc * B:(pc + 1) * B],
                        start=(pc == 0), stop=(pc == 3))
                nc.scalar.copy(acc[:, g * B:(g + 1) * B], accg[:])

            M4 = emit_M(0)
            Eg_prev = None
            for g in range(NG):
                Mb4 = wp.tile([128, 512], bf16, tag="Mb")
                nc.scalar.copy(Mb4[:], M4[:])

                distg = wp.tile([128, 512], f32, tag="dist")
                for pc in range(4):
                    for h in range(2):
                        psD = ps_d.tile([128, 1024], f32, tag="psD")
                        for i2 in range(2):
                            ncn = 2 * h + i2
                            nc.tensor.matmul(
                                psD[:, i2 * 512:(i2 + 1) * 512],
                                sgn_sb[32 * ncn:32 * (ncn + 1),
                                       pc * 128:(pc + 1) * 128],
                                Mb4[32 * ncn:32 * (ncn + 1), :],
                                start=True, stop=True,
                                tile_position=(32 * ncn, 0))
                        nc.vector.tensor_reduce(
                            distg[:, pc * 128 + h * 64:pc * 128 + (h + 1) * 64],
                            psD[:].rearrange("p (s f) -> p s f", f=F),
                            axis=mybir.AxisListType.X, op=ALU.add,
                            apply_absolute_value=True)
                if Eg_prev is not None:
                    emit_accg(g - 1, Eg_prev)
                if g + 1 < NG:
                    M4 = emit_M(g + 1)
                Eg = wp.tile([128, 512], bf16, tag="E")
                nc.scalar.activation(Eg[:], distg[:], AFT.Exp, scale=-1.0 / TS)
                Eg_prev = Eg
            emit_accg(NG - 1, Eg_prev)

            # ---- Stage F: deconv + store
            # acc[32*jm + ci, (r*4+jd)*32 + b] already has ci on partitions.
            # Weights are zero-padded to full 128 rows per (jm, v) so each
            # matmul is a plain full-height one (no row tiling: the stride-4
            # column interleave would put multiple row-groups in one psum
            # bank, which the HW rejects).
            acc_r = acc[:].rearrange("p (r jd b) -> p r jd b", r=2, jd=4)
            for r in range(2):
                for bh in range(2):
                    yps_t = ps_d.tile([128, 1024], f32, tag="psD")
                    yps = yps_t[:].rearrange(
                        "p (b jd jm v) -> p jm v b jd", jd=4, jm=4, v=4)
                    for zh in range(2):
                        nc.tensor.matmul(
                            yps_t[:, zh * 512:(zh + 1) * 512],
                            zrow_sb[:], sgn_sb[0:1, :],
                            start=True, stop=True)
                    for jm in range(4):
                        for v in range(4):
                            nc.tensor.matmul(
                                yps[:, jm, v],
                                wd_sb[:, (jm * 4 + v) * 128:
                                      (jm * 4 + v + 1) * 128],
                                acc_r[:, r, :, bh * 16:(bh + 1) * 16]
                                .rearrange("p jd b -> p b jd"),
                                start=False, stop=True)
                    yst = wp.tile([128, 1024], bf16, tag="yst")
                    eng = nc.vector.tensor_copy if bh == 0 else nc.scalar.copy
                    eng(yst[:], yps_t[:])
                    nc.sync.dma_start(
                        y[r, :, bh * 1024:(bh + 1) * 1024], yst[:])

    nc.finalize()
    return nc


def _host_prep(x, w_conv, T, w_deconv):
    """Build the 8 per-core input maps."""
    bf = ml_dtypes.bfloat16
    f8 = ml_dtypes.float8_e4m3

    # x: per core (128 ch, (b, rs, ij)) with x[b,ch,4i+r,4j+s] at
    # col = b*4096 + (r*4+s)*256 + i*16 + j; x32 -> fp8
    xq = np.clip(np.asarray(x, np.float32) * XS, -240, 240)
    xq = xq.reshape(B, IN_FLT, 16, 4, 16, 4)          # b ch i r j s
    xq = np.ascontiguousarray(xq.transpose(1, 0, 3, 5, 2, 4)).astype(f8)
    # now (ch, b, r, s, i, j)

    # conv weights: lhsT[(c), (idx,o)] = w_conv[o, c, r, s] / XS
    wc_host = np.ascontiguousarray(
        np.transpose(w_conv / XS, (1, 2, 3, 0)).reshape(IN_FLT, 48)).astype(bf)

    # deconv weights zero-padded to 128 rows:
    # wd4[32*jm + ci, (jm*4 + v)*128 + u*32 + co] = w_deconv[co, ci, u, v]
    wd_small = np.transpose(w_deconv, (1, 3, 2, 0)).reshape(OC, 4, 128)  # ci,v,uc
    wd_host = np.zeros((128, 2048), np.float32)
    for jm in range(4):
        for v in range(4):
            wd_host[32 * jm:32 * (jm + 1), (jm * 4 + v) * 128:
                    (jm * 4 + v + 1) * 128] = wd_small[:, v, :]
    wd_host = wd_host.astype(bf)

    eye_host = np.eye(B, dtype=np.float32).astype(bf)

    # pairwise sign matrix (496 pairs padded to 512) and incidence
    pairs = [(a, b) for a in range(B) for b in range(a + 1, B)]
    sgn_host = np.zeros((128, 512), np.float32)
    inc_host = np.zeros((128, 128), np.float32)
    for p, (a, b) in enumerate(pairs):
        for i in range(4):
            sgn_host[32 * i + a, p] = 1.0
            sgn_host[32 * i + b, p] = -1.0
        inc_host[p % 128, (p // 128) * B + a] = 1.0
        inc_host[p % 128, (p // 128) * B + b] = 1.0
    sgn_host = sgn_host.astype(bf)
    inc_host = inc_host.astype(bf)

    # T: (768, 8192, 16) f32; din=(k,p), d=(ch,i,j), i=(core,r), j=(jd,jm)
    # per-core dram layout [g=(r,jd)][p][k][jm][ch][f], x1024 -> fp8
    T8 = np.asarray(T, np.float32).reshape(KCH, 128, OC, 8, 2, 4, 4, F)
    T8 = np.clip(T8 * TS, -240, 240)
    # (k p ch c r jd jm f) -> (c, r, jd, p, k, jm, ch, f)
    T8 = np.ascontiguousarray(T8.transpose(3, 4, 5, 1, 0, 6, 2, 7)).astype(f8)
    T8 = T8.reshape(N_CORES, NG, 128, KCH * 2048)

    in_maps = []
    for c in range(N_CORES):
        in_maps.append({
            "zrow": np.zeros((1, 128), bf),
            "xc8": np.ascontiguousarray(
                xq[:, BC * c:BC * (c + 1)]).reshape(IN_FLT, BC * N * N),
            "tsh": T8[c],
            "wc": wc_host,
            "wd4": wd_host,
            "sgn4": sgn_host,
            "inc": inc_host,
            "eye": eye_host,
        })
    return in_maps


def _get_nc():
    if "nc" not in _CACHE:
        _CACHE["nc"] = _build_nc()
    return _CACHE["nc"]


def run(inputs, trace=False, trace_kwargs=None):
    """Run on hardware; returns (full_output, BassKernelResults)."""
    from concourse.bass_utils import run_bass_kernel_spmd
    nc = _get_nc()
    in_maps = _host_prep(inputs["x"], inputs["w_conv"], inputs["T"],
                         inputs["w_deconv"])
    res = run_bass_kernel_spmd(nc, in_maps, list(range(N_CORES)), trace=trace,
                               **(trace_kwargs or {}))
    x = np.asarray(inputs["x"], dtype=np.float32)
    full = np.empty((B, IN_FLT + OC, N, N), np.float32)
    full[:, :IN_FLT] = x
    for c in range(N_CORES):
        yv = np.asarray(res.results[c]["y"], dtype=np.float32).reshape(2, 4, OC, B, N)  # (r, u, co, b, w)
        for r in range(2):
            # dest dims are (b, co, u, w)
            full[:, IN_FLT:, 8 * c + 4 * r:8 * c + 4 * r + 4, :] = \
                yv[r].transpose(2, 1, 0, 3)
    return full, res


def kernel(**inputs) -> np.ndarray:
    out, _ = run(inputs, trace=False)
    return out


# revision 3
# speedup vs baseline: 1.0786x; 1.0786x over previous
"""Trainium2 Bass kernel for MinibatchDiscrimination2d (v2, fp8 + PE tiling).

Full computation:
  x (32,128,64,64) --conv s4--> x_r (32,3,16,16)
  M = x_r @ T  -> (32, 8192, 16)
  dist[b1,b2,d] = sum_f |M[b1,d,f]-M[b2,d,f]|
  out[b,d] = sum_b2 exp(-dist) - 1 -> (32,32,16,16)
  out_a = deconv s4 (32,32,64,64); return concat([x, out_a], ch)

Sharding over 8 cores (core c owns t-rows 2c, 2c+1 of the 16x16 spatial
grid -> 1024 of the 8192 d values, all 32 samples; conv data-parallel
over b with an AllGather of the tiny x_r).

Per-core d index: d_local = (r, j, ch) with r in 2, j in 16, ch in 32.
Grouping: g = r*4 + j//4 (8 groups of 128), ncn = j%4, s32 = ch.
s_local within g = ncn*32 + ch.

Numerics: T is scaled x1024 and sent as fp8 e4m3 (TRN variant, max 240);
x is scaled x32 to fp8 with conv weights pre-divided by 32. M is kept as
1024*M in bf16; the exp activation uses scale = -1/1024. Host-emulated
global rel err ~5e-4 (gate 2e-2).

Engine structure per g (software-pipelined: accg deferred one g so the
PE never stalls on exp; M(g+1) emitted before accg(g)):
  M: col-tiled x4 matmuls (tile_position=(0,32j)), T streamed fp8,
     psum M4 (128=(ncn,b), 512) accumulated over 6 din chunks.
  D: row-tiled x4 matmuls (tile_position=(32*ncn,0)), sgn weights,
     each reading Mb4[32ncn:32ncn+32, :]. Each row group MUST land in
     its own psum bank: two (128,1024) 2-bank psum tiles per pc.
  dist: vector fused abs-reduce over f=16 (psum f32, 1x - the
     throughput floor of this kernel, ~73us busy).
  E=exp(-dist/1024) on scalar; acc += E^T @ inc on PE.
Deconv: psum zeroed by PE matmuls against a zero row (vector memset is
slower), then 16 accumulating matmuls (start=False) write interleaved
stride-4 columns; per-matmul start=True is NOT safe for multiple
writers within one psum bank. Copy to bf16 sbuf, DMA out; host
reassembles (y[r] rows = u*32+oc, cols = b*64+w).

Known timing structure (~170-190us, +-12us run jitter):
  ~7-18us  x load + conv;  ~18-70us AllGather of x_r (fixed ~45us
  collective latency in this runtime, payload-independent);
  ~70-155us g-loop (vector-saturated);  ~155-168us deconv tail.
"""

import numpy as np
import ml_dtypes

N_CORES = 8
B, IN_FLT, N = 32, 128, 64
K = 4
T_SP = 16
OC = 32
F = 16
D_IN = 768
BC = B // N_CORES          # 4 samples per core (conv data-parallel)
DSH = 1024                 # d per core
NG = 8                     # dgroups of 128 s-values
KCH = 6                    # 128-row din chunks
TS = 1024.0                # T scale into fp8
XS = 32.0                  # x scale into fp8

_CACHE = {}


def _build_nc():
    import concourse.bacc as bacc
    import concourse.mybir as mybir
    import concourse.tile as tile

    f32 = mybir.dt.float32
    bf16 = mybir.dt.bfloat16
    fp8 = mybir.dt.float8e4
    AFT = mybir.ActivationFunctionType
    ALU = mybir.AluOpType

    nc = bacc.Bacc("TRN2", target_bir_lowering=False, debug=False,
                   num_devices=N_CORES)

    xc8 = nc.dram_tensor("xc8", [IN_FLT, BC * N * N], fp8, kind="ExternalInput")
    tsh = nc.dram_tensor("tsh", [NG, 128, KCH * 2048], fp8, kind="ExternalInput")
    wc = nc.dram_tensor("wc", [IN_FLT, 48], bf16, kind="ExternalInput")
    wd4 = nc.dram_tensor("wd4", [128, 2048], bf16, kind="ExternalInput")
    sgn4 = nc.dram_tensor("sgn4", [128, 512], bf16, kind="ExternalInput")
    inc = nc.dram_tensor("inc", [128, 128], bf16, kind="ExternalInput")
    eye = nc.dram_tensor("eye", [B, B], bf16, kind="ExternalInput")
    zrow = nc.dram_tensor("zrow", [1, 128], bf16, kind="ExternalInput")
    y = nc.dram_tensor("y", [2, 128, 2048], bf16, kind="ExternalOutput")

    with tile.TileContext(nc) as tc:
        with tc.tile_pool(name="const", bufs=1) as constp, \
             tc.tile_pool(name="dram", bufs=1, space="DRAM") as dram, \
             tc.tile_pool(name="xb", bufs=1) as xbp, \
             tc.tile_pool(name="Tp", bufs=2) as Tp, \
             tc.tile_pool(name="work", bufs=2) as wp, \
             tc.tile_pool(name="persist", bufs=1) as pp, \
             tc.tile_pool(name="ps_cv", bufs=1, space="PSUM") as ps_cv, \
             tc.tile_pool(name="ps_m", bufs=2, space="PSUM") as ps_m, \
             tc.tile_pool(name="ps_d", bufs=2, space="PSUM") as ps_d, \
             tc.tile_pool(name="ps_a", bufs=1, space="PSUM") as ps_a:

            wc_sb = constp.tile([IN_FLT, 48], bf16)
            nc.scalar.dma_start(wc_sb[:], wc[:])
            wd_sb = constp.tile([128, 2048], bf16)
            nc.scalar.dma_start(wd_sb[:], wd4[:])
            sgn_sb = constp.tile([128, 512], bf16)
            nc.scalar.dma_start(sgn_sb[:], sgn4[:])
            inc_sb = constp.tile([128, 128], bf16)
            nc.scalar.dma_start(inc_sb[:], inc[:])
            eye_sb = constp.tile([B, B], bf16)
            nc.scalar.dma_start(eye_sb[:], eye[:])
            zrow_sb = constp.tile([1, 128], bf16)
            nc.scalar.dma_start(zrow_sb[:], zrow[:])

            # ---- x shard load (fp8, split per b-pair) + T prefetch
            x8 = xbp.tile([IN_FLT, BC * N * N], fp8, tag="x8")
            for half in range(2):
                nc.sync.dma_start(x8[:, half * 8192:(half + 1) * 8192],
                                  xc8[:, half * 8192:(half + 1) * 8192])

            Ts = []
            for g in range(NG):
                Tt = Tp.tile([128, KCH * 2048], fp8, tag="T")
                nc.sync.dma_start(Tt[:], tsh[g, :, :])
                Ts.append(Tt)

            # ---- Stage A: conv.  x8 cols = (b(4), rs(16), ij(256))
            xrl = pp.tile([3, BC * 256], bf16)       # cols = (b, i, j)
            x8_rs = x8[:].rearrange(
                "c (b rs ij) -> c b rs ij", b=BC, rs=16)
            for half in range(2):
                psc_t = ps_cv.tile([128, 512], f32, tag="cv")
                psc = psc_t[:3, :]
                for idx in range(16):
                    nc.tensor.matmul(
                        psc[:].rearrange("p (b ij) -> p b ij", b=2),
                        wc_sb[:, idx * 3:idx * 3 + 3],
                        x8_rs[:, half * 2:(half + 1) * 2, idx],
                        start=(idx == 0), stop=(idx == 15))
                nc.scalar.copy(xrl[:, half * 512:(half + 1) * 512], psc[:])

            ag_in = dram.tile([BC, D_IN], bf16)
            ag_out = dram.tile([B, D_IN], bf16)
            nc.gpsimd.dma_start(
                ag_in[:].rearrange("b (c ij) -> c b ij", c=3),
                xrl[:].rearrange("c (b ij) -> c b ij", b=BC))
            nc.gpsimd.collective_compute(
                "AllGather", ALU.bypass,
                replica_groups=[list(range(N_CORES))],
                ins=[ag_in.opt()], outs=[ag_out.opt()])

            # ---- Stage B: x_r^T via eye-matmul (lhsT = xr slice, rhs = eye)
            xr_all = pp.tile([B, D_IN], bf16)
            nc.gpsimd.dma_start(xr_all[:], ag_out[:])
            xrT = pp.tile([128, KCH * B], bf16)
            for k in range(KCH):
                pst_t = ps_cv.tile([128, 512], f32, tag="cv")
                pst = pst_t[:, :B]
                nc.tensor.matmul(pst[:], xr_all[:, k * 128:(k + 1) * 128],
                                 eye_sb[:], start=True, stop=True)
                nc.scalar.copy(xrT[:, k * B:(k + 1) * B], pst[:])

            acc = pp.tile([128, NG * B], bf16)       # (s_local, (g, b))

            # ---- Stages C/D/E fused per dgroup g
            def emit_M(g):
                Tt = Ts[g]
                M4 = ps_m.tile([128, 512], f32, tag="M4")
                for k in range(KCH):
                    for j in range(4):
                        nc.tensor.matmul(
                            M4[32 * j:32 * (j + 1), :],
                            xrT[:, k * B:(k + 1) * B],
                            Tt[:, k * 2048 + j * 512:k * 2048 + (j + 1) * 512],
                            start=(k == 0), stop=(k == KCH - 1),
                            tile_position=(0, 32 * j))
                return M4

            def emit_accg(g, Eg):
                accg_t = ps_a.tile([128, 512], f32, tag="accg")
                accg = accg_t[:, :B]
                for pc in range(4):
                    nc.tensor.matmul(
                        accg[:], Eg[:, pc * 128:(pc + 1) * 128],
                        inc_sb[:, pc * B:(pc + 1) * B],
                        start=(pc == 0), stop=(pc == 3))
                nc.scalar.copy(acc[:, g * B:(g + 1) * B], accg[:])

            M4 = emit_M(0)
            Eg_prev = None
            for g in range(NG):
                Mb4 = wp.tile([128, 512], bf16, tag="Mb")
                nc.scalar.copy(Mb4[:], M4[:])

                distg = wp.tile([128, 512], f32, tag="dist")
                for pc in range(4):
                    for h in range(2):
                        psD = ps_d.tile([128, 1024], f32, tag="psD")
                        for i2 in range(2):
                            ncn = 2 * h + i2
                            nc.tensor.matmul(
                                psD[:, i2 * 512:(i2 + 1) * 512],
                                sgn_sb[32 * ncn:32 * (ncn + 1),
                                       pc * 128:(pc + 1) * 128],
                                Mb4[32 * ncn:32 * (ncn + 1), :],
                                start=True, stop=True,
                                tile_position=(32 * ncn, 0))
                        nc.vector.tensor_reduce(
                            distg[:, pc * 128 + h * 64:pc * 128 + (h + 1) * 64],
                            psD[:].rearrange("p (s f) -> p s f", f=F),
                            axis=mybir.AxisListType.X, op=ALU.add,
                            apply_absolute_value=True)
                if Eg_prev is not None:
                    emit_accg(g - 1, Eg_prev)
                if g + 1 < NG:
                    M4 = emit_M(g + 1)
                Eg = wp.tile([128, 512], bf16, tag="E")
                nc.scalar.activation(Eg[:], distg[:], AFT.Exp, scale=-1.0 / TS)
                Eg_prev = Eg
            emit_accg(NG - 1, Eg_prev)

            # ---- Stage F: deconv + store
            # acc[32*jm + ci, (r*4+jd)*32 + b] already has ci on partitions.
            # Weights are zero-padded to full 128 rows per (jm, v) so each
            # matmul is a plain full-height one (no row tiling: the stride-4
            # column interleave would put multiple row-groups in one psum
            # bank, which the HW rejects).
            acc_r = acc[:].rearrange("p (r jd b) -> p r jd b", r=2, jd=4)
            for r in range(2):
                for bh in range(2):
                    yps_t = ps_d.tile([128, 1024], f32, tag="psD")
                    yps = yps_t[:].rearrange(
                        "p (b jd jm v) -> p jm v b jd", jd=4, jm=4, v=4)
                    for zh in range(2):
                        nc.tensor.matmul(
                            yps_t[:, zh * 512:(zh + 1) * 512],
                            zrow_sb[:], sgn_sb[0:1, :],
                            start=True, stop=True)
                    for jm in range(4):
                        for v in range(4):
                            nc.tensor.matmul(
                                yps[:, jm, v],
                                wd_sb[:, (jm * 4 + v) * 128:
                                      (jm * 4 + v + 1) * 128],
                                acc_r[:, r, :, bh * 16:(bh + 1) * 16]
                                .rearrange("p jd b -> p b jd"),
                                start=False, stop=True)
                    yst = wp.tile([128, 1024], bf16, tag="yst")
                    eng = nc.vector.tensor_copy if bh == 0 else nc.scalar.copy
                    eng(yst[:], yps_t[:])
                    nc.sync.dma_start(
                        y[r, :, bh * 1024:(bh + 1) * 1024], yst[:])

    nc.finalize()
    return nc


def _host_prep(x, w_conv, T, w_deconv):
    """Build the 8 per-core input maps."""
    bf = ml_dtypes.bfloat16
    f8 = ml_dtypes.float8_e4m3

    # x: per core (128 ch, (b, rs, ij)) with x[b,ch,4i+r,4j+s] at
    # col = b*4096 + (r*4+s)*256 + i*16 + j; x32 -> fp8
    xq = np.clip(np.asarray(x, np.float32) * XS, -240, 240)
    xq = xq.reshape(B, IN_FLT, 16, 4, 16, 4)          # b ch i r j s
    xq = np.ascontiguousarray(xq.transpose(1, 0, 3, 5, 2, 4)).astype(f8)
    # now (ch, b, r, s, i, j)

    # conv weights: lhsT[(c), (idx,o)] = w_conv[o, c, r, s] / XS
    wc_host = np.ascontiguousarray(
        np.transpose(w_conv / XS, (1, 2, 3, 0)).reshape(IN_FLT, 48)).astype(bf)

    # deconv weights zero-padded to 128 rows:
    # wd4[32*jm + ci, (jm*4 + v)*128 + u*32 + co] = w_deconv[co, ci, u, v]
    wd_small = np.transpose(w_deconv, (1, 3, 2, 0)).reshape(OC, 4, 128)  # ci,v,uc
    wd_host = np.zeros((128, 2048), np.float32)
    for jm in range(4):
        for v in range(4):
            wd_host[32 * jm:32 * (jm + 1), (jm * 4 + v) * 128:
                    (jm * 4 + v + 1) * 128] = wd_small[:, v, :]
    wd_host = wd_host.astype(bf)

    eye_host = np.eye(B, dtype=np.float32).astype(bf)

    # pairwise sign matrix (496 pairs padded to 512) and incidence
    pairs = [(a, b) for a in range(B) for b in range(a + 1, B)]
    sgn_host = np.zeros((128, 512), np.float32)
    inc_host = np.zeros((128, 128), np.float32)
    for p, (a, b) in enumerate(pairs):
        for i in range(4):
            sgn_host[32 * i + a, p] = 1.0
            sgn_host[32 * i + b, p] = -1.0
        inc_host[p % 128, (p // 128) * B + a] = 1.0
        inc_host[p % 128, (p // 128) * B + b] = 1.0
    sgn_host = sgn_host.astype(bf)
    inc_host = inc_host.astype(bf)

    # T: (768, 8192, 16) f32; din=(k,p), d=(ch,i,j), i=(core,r), j=(jd,jm)
    # per-core dram layout [g=(r,jd)][p][k][jm][ch][f], x1024 -> fp8
    T8 = np.asarray(T, np.float32).reshape(KCH, 128, OC, 8, 2, 4, 4, F)
    T8 = np.clip(T8 * TS, -240, 240)
    # (k p ch c r jd jm f) -> (c, r, jd, p, k, jm, ch, f)
    T8 = np.ascontiguousarray(T8.transpose(3, 4, 5, 1, 0, 6, 2, 7)).astype(f8)
    T8 = T8.reshape(N_CORES, NG, 128, KCH * 2048)

    in_maps = []
    for c in range(N_CORES):
        in_maps.append({
            "zrow": np.zeros((1, 128), bf),
            "xc8": np.ascontiguousarray(
                xq[:, BC * c:BC * (c + 1)]).reshape(IN_FLT, BC * N * N),
            "tsh": T8[c],
            "wc": wc_host,
            "wd4": wd_host,
            "sgn4": sgn_host,
            "inc": inc_host,
            "eye": eye_host,
        })
    return in_maps


def _get_nc():
    if "nc" not in _CACHE:
        _CACHE["nc"] = _build_nc()
    return _CACHE["nc"]


def run(inputs, trace=False, trace_kwargs=None):
    """Run on hardware; returns (full_output, BassKernelResults)."""
    from concourse.bass_utils import run_bass_kernel_spmd
    nc = _get_nc()
    in_maps = _host_prep(inputs["x"], inputs["w_conv"], inputs["T"],
                         inputs["w_deconv"])
    res = run_bass_kernel_spmd(nc, in_maps, list(range(N_CORES)), trace=trace,
                               **(trace_kwargs or {}))
    x = np.asarray(inputs["x"], dtype=np.float32)
    full = np.empty((B, IN_FLT + OC, N, N), np.float32)
    full[:, :IN_FLT] = x
    for c in range(N_CORES):
        yv = np.asarray(res.results[c]["y"], dtype=np.float32).reshape(2, 4, OC, B, N)  # (r, u, co, b, w)
        for r in range(2):
            # dest dims are (b, co, u, w)
            full[:, IN_FLT:, 8 * c + 4 * r:8 * c + 4 * r + 4, :] = \
                yv[r].transpose(2, 1, 0, 3)
    return full, res


def kernel(**inputs) -> np.ndarray:
    out, _ = run(inputs, trace=False)
    return out


# revision 5
# speedup vs baseline: 1.1917x; 1.1049x over previous
"""Trainium2 Bass kernel for MinibatchDiscrimination2d (v2, fp8 + PE tiling).

Full computation:
  x (32,128,64,64) --conv s4--> x_r (32,3,16,16)
  M = x_r @ T  -> (32, 8192, 16)
  dist[b1,b2,d] = sum_f |M[b1,d,f]-M[b2,d,f]|
  out[b,d] = sum_b2 exp(-dist) - 1 -> (32,32,16,16)
  out_a = deconv s4 (32,32,64,64); return concat([x, out_a], ch)

Sharding over 8 cores (core c owns t-rows 2c, 2c+1 of the 16x16 spatial
grid -> 1024 of the 8192 d values, all 32 samples; conv data-parallel
over b with an AllGather of the tiny x_r).

Per-core d index: d_local = (r, j, ch) with r in 2, j in 16, ch in 32.
Grouping: g = r*4 + j//4 (8 groups of 128), ncn = j%4, s32 = ch.
s_local within g = ncn*32 + ch.

Numerics: T is scaled x1024 and sent as fp8 e4m3 (TRN variant, max 240);
x is scaled x32 to fp8 with conv weights pre-divided by 32. M is kept as
1024*M in bf16; the exp activation uses scale = -1/1024. Host-emulated
global rel err ~5e-4 (gate 2e-2).

Engine structure per g (software-pipelined: accg deferred one g so the
PE never stalls on exp; M(g+1) emitted before accg(g)):
  M: col-tiled x4 matmuls (tile_position=(0,32j)), T streamed fp8,
     psum M4 (128=(ncn,b), 512) accumulated over 6 din chunks.
  D: row-tiled x4 matmuls (tile_position=(32*ncn,0)), sgn weights,
     each reading Mb4[32ncn:32ncn+32, :]. Each row group MUST land in
     its own psum bank: two (128,1024) 2-bank psum tiles per pc.
  dist: vector fused abs-reduce over f=16 (psum f32, 1x - the
     throughput floor of this kernel, ~73us busy).
  E=exp(-dist/1024) on scalar; acc += E^T @ inc on PE.
Deconv: psum zeroed by PE matmuls against a zero row (vector memset is
slower), then 16 accumulating matmuls (start=False) write interleaved
stride-4 columns; per-matmul start=True is NOT safe for multiple
writers within one psum bank. Copy to bf16 sbuf, DMA out; host
reassembles (y[r] rows = u*32+oc, cols = b*64+w).

Known timing structure (~170-190us, +-12us run jitter):
  ~7-18us  x load + conv;  ~18-70us AllGather of x_r (fixed ~45us
  collective latency in this runtime, payload-independent);
  ~70-155us g-loop (vector-saturated);  ~155-168us deconv tail.
"""

import numpy as np
import ml_dtypes

N_CORES = 8
B, IN_FLT, N = 32, 128, 64
K = 4
T_SP = 16
OC = 32
F = 16
D_IN = 768
BC = B // N_CORES          # 4 samples per core (conv data-parallel)
DSH = 1024                 # d per core
NG = 8                     # dgroups of 128 s-values
KCH = 6                    # 128-row din chunks
TS = 1024.0                # T scale into fp8
XS = 32.0                  # x scale into fp8

_CACHE = {}


def _build_nc():
    import concourse.bacc as bacc
    import concourse.mybir as mybir
    import concourse.tile as tile

    f32 = mybir.dt.float32
    bf16 = mybir.dt.bfloat16
    fp8 = mybir.dt.float8e4
    AFT = mybir.ActivationFunctionType
    ALU = mybir.AluOpType

    nc = bacc.Bacc("TRN2", target_bir_lowering=False, debug=False,
                   num_devices=N_CORES)

    xc8 = nc.dram_tensor("xc8", [IN_FLT, BC * N * N], fp8, kind="ExternalInput")
    tsh = nc.dram_tensor("tsh", [NG, 128, KCH * 2048], fp8, kind="ExternalInput")
    wc = nc.dram_tensor("wc", [IN_FLT, 48], bf16, kind="ExternalInput")
    wd4 = nc.dram_tensor("wd4", [128, 2048], bf16, kind="ExternalInput")
    sgn4 = nc.dram_tensor("sgn4", [128, 512], bf16, kind="ExternalInput")
    inc = nc.dram_tensor("inc", [128, 128], bf16, kind="ExternalInput")
    eye = nc.dram_tensor("eye", [B, B], bf16, kind="ExternalInput")
    zrow = nc.dram_tensor("zrow", [1, 128], bf16, kind="ExternalInput")
    y = nc.dram_tensor("y", [2, 128, 2048], bf16, kind="ExternalOutput")

    with tile.TileContext(nc) as tc:
        with tc.tile_pool(name="const", bufs=1) as constp, \
             tc.tile_pool(name="dram", bufs=1, space="DRAM") as dram, \
             tc.tile_pool(name="xb", bufs=1) as xbp, \
             tc.tile_pool(name="Tp", bufs=3) as Tp, \
             tc.tile_pool(name="work", bufs=2) as wp, \
             tc.tile_pool(name="persist", bufs=1) as pp, \
             tc.tile_pool(name="ps_cv", bufs=1, space="PSUM") as ps_cv, \
             tc.tile_pool(name="ps_m", bufs=2, space="PSUM") as ps_m, \
             tc.tile_pool(name="ps_d", bufs=2, space="PSUM") as ps_d, \
             tc.tile_pool(name="ps_a", bufs=1, space="PSUM") as ps_a:

            wc_sb = constp.tile([IN_FLT, 48], bf16)
            nc.scalar.dma_start(wc_sb[:], wc[:])
            wd_sb = constp.tile([128, 2048], bf16)
            nc.scalar.dma_start(wd_sb[:], wd4[:])
            sgn_sb = constp.tile([128, 512], bf16)
            nc.scalar.dma_start(sgn_sb[:], sgn4[:])
            inc_sb = constp.tile([128, 128], bf16)
            nc.scalar.dma_start(inc_sb[:], inc[:])
            eye_sb = constp.tile([B, B], bf16)
            nc.scalar.dma_start(eye_sb[:], eye[:])
            zrow_sb = constp.tile([1, 128], bf16)
            nc.scalar.dma_start(zrow_sb[:], zrow[:])

            # ---- x shard load (fp8, split per b-pair) + T prefetch
            x8 = xbp.tile([IN_FLT, BC * N * N], fp8, tag="x8")
            for half in range(2):
                nc.sync.dma_start(x8[:, half * 8192:(half + 1) * 8192],
                                  xc8[:, half * 8192:(half + 1) * 8192])

            Ts = []
            for g in range(NG):
                Tt = Tp.tile([128, KCH * 2048], fp8, tag="T")
                nc.sync.dma_start(Tt[:], tsh[g, :, :])
                Ts.append(Tt)

            # ---- Stage A: conv.  x8 cols = (b(4), rs(16), ij(256))
            xrl = pp.tile([3, BC * 256], bf16)       # cols = (b, i, j)
            x8_rs = x8[:].rearrange(
                "c (b rs ij) -> c b rs ij", b=BC, rs=16)
            for half in range(2):
                psc_t = ps_cv.tile([128, 512], f32, tag="cv")
                psc = psc_t[:3, :]
                for idx in range(16):
                    for b2 in range(2):
                        nc.tensor.matmul(
                            psc[:, b2 * 256:(b2 + 1) * 256],
                            wc_sb[:, idx * 3:idx * 3 + 3],
                            x8_rs[:, half * 2 + b2, idx],
                            start=(idx == 0), stop=(idx == 15))
                nc.scalar.copy(xrl[:, half * 512:(half + 1) * 512], psc[:])

            ag_in = dram.tile([BC, D_IN], bf16)
            ag_out = dram.tile([B, D_IN], bf16)
            nc.gpsimd.dma_start(
                ag_in[:].rearrange("b (c ij) -> c b ij", c=3),
                xrl[:].rearrange("c (b ij) -> c b ij", b=BC))
            nc.gpsimd.collective_compute(
                "AllGather", ALU.bypass,
                replica_groups=[list(range(N_CORES))],
                ins=[ag_in.opt()], outs=[ag_out.opt()])

            # ---- Stage B: x_r^T via eye-matmul (lhsT = xr slice, rhs = eye)
            xr_all = pp.tile([B, D_IN], bf16)
            nc.gpsimd.dma_start(xr_all[:], ag_out[:])
            xrT = pp.tile([128, KCH * B], bf16)
            for k in range(KCH):
                pst_t = ps_cv.tile([128, 512], f32, tag="cv")
                pst = pst_t[:, :B]
                nc.tensor.matmul(pst[:], xr_all[:, k * 128:(k + 1) * 128],
                                 eye_sb[:], start=True, stop=True)
                nc.scalar.copy(xrT[:, k * B:(k + 1) * B], pst[:])

            acc = pp.tile([128, NG * B], bf16)       # (s_local, (g, b))

            # ---- Stage F emitted via emit_deconv(r) (r=0 mid-loop)
            acc_r = acc[:].rearrange("p (r jd b) -> p r jd b", r=2, jd=4)
            def emit_deconv(r):
                for bh in range(2):
                    yps_t = ps_d.tile([128, 1024], f32, tag="psD")
                    yps = yps_t[:].rearrange(
                        "p (b jd jm v) -> p jm v b jd", jd=4, jm=4, v=4)
                    for zh in range(2):
                        nc.tensor.matmul(
                            yps_t[:, zh * 512:(zh + 1) * 512],
                            zrow_sb[:], sgn_sb[0:1, :],
                            start=True, stop=True)
                    for jm in range(4):
                        for v in range(4):
                            nc.tensor.matmul(
                                yps[:, jm, v],
                                wd_sb[:, (jm * 4 + v) * 128:
                                      (jm * 4 + v + 1) * 128],
                                acc_r[:, r, :, bh * 16:(bh + 1) * 16]
                                .rearrange("p jd b -> p b jd"),
                                start=False, stop=True)
                    yst = wp.tile([128, 1024], bf16, tag="yst")
                    eng = nc.vector.tensor_copy if bh == 0 else nc.scalar.copy
                    eng(yst[:], yps_t[:])
                    nc.sync.dma_start(
                        y[r, :, bh * 1024:(bh + 1) * 1024], yst[:])

            # ---- Stages C/D/E fused per dgroup g
            def emit_M(g):
                Tt = Ts[g]
                M4 = ps_m.tile([128, 512], f32, tag="M4")
                for k in range(KCH):
                    for j in range(4):
                        nc.tensor.matmul(
                            M4[32 * j:32 * (j + 1), :],
                            xrT[:, k * B:(k + 1) * B],
                            Tt[:, k * 2048 + j * 512:k * 2048 + (j + 1) * 512],
                            start=(k == 0), stop=(k == KCH - 1),
                            tile_position=(0, 32 * j))
                return M4

            def emit_accg(g, Eg):
                accg_t = ps_a.tile([128, 512], f32, tag="accg")
                accg = accg_t[:, :B]
                for pc in range(4):
                    nc.tensor.matmul(
                        accg[:], Eg[:, pc * 128:(pc + 1) * 128],
                        inc_sb[:, pc * B:(pc + 1) * B],
                        start=(pc == 0), stop=(pc == 3))
                nc.scalar.copy(acc[:, g * B:(g + 1) * B], accg[:])

            M4 = emit_M(0)
            Eg_prev = None
            for g in range(NG):
                Mb4 = wp.tile([128, 512], bf16, tag="Mb")
                nc.scalar.copy(Mb4[:], M4[:])

                distg = wp.tile([128, 512], f32, tag="dist")
                for pc in range(4):
                    for h in range(2):
                        psD = ps_d.tile([128, 1024], f32, tag="psD")
                        for i2 in range(2):
                            ncn = 2 * h + i2
                            nc.tensor.matmul(
                                psD[:, i2 * 512:(i2 + 1) * 512],
                                sgn_sb[32 * ncn:32 * (ncn + 1),
                                       pc * 128:(pc + 1) * 128],
                                Mb4[32 * ncn:32 * (ncn + 1), :],
                                start=True, stop=True,
                                tile_position=(32 * ncn, 0))
                        nc.vector.tensor_reduce(
                            distg[:, pc * 128 + h * 64:pc * 128 + (h + 1) * 64],
                            psD[:].rearrange("p (s f) -> p s f", f=F),
                            axis=mybir.AxisListType.X, op=ALU.add,
                            apply_absolute_value=True)
                if Eg_prev is not None:
                    emit_accg(g - 1, Eg_prev)
                if g == 5:
                    emit_deconv(0)
                if g + 1 < NG:
                    M4 = emit_M(g + 1)
                Eg = wp.tile([128, 512], bf16, tag="E")
                nc.scalar.activation(Eg[:], distg[:], AFT.Exp, scale=-1.0 / TS)
                Eg_prev = Eg
            emit_accg(NG - 1, Eg_prev)

            emit_deconv(1)

    nc.finalize()
    return nc


def _host_prep(x, w_conv, T, w_deconv):
    """Build the 8 per-core input maps."""
    bf = ml_dtypes.bfloat16
    f8 = ml_dtypes.float8_e4m3

    # x: per core (128 ch, (b, rs, ij)) with x[b,ch,4i+r,4j+s] at
    # col = b*4096 + (r*4+s)*256 + i*16 + j; x32 -> fp8
    xq = np.clip(np.asarray(x, np.float32) * XS, -240, 240)
    xq = xq.reshape(B, IN_FLT, 16, 4, 16, 4)          # b ch i r j s
    xq = np.ascontiguousarray(xq.transpose(1, 0, 3, 5, 2, 4)).astype(f8)
    # now (ch, b, r, s, i, j)

    # conv weights: lhsT[(c), (idx,o)] = w_conv[o, c, r, s] / XS
    wc_host = np.ascontiguousarray(
        np.transpose(w_conv / XS, (1, 2, 3, 0)).reshape(IN_FLT, 48)).astype(bf)

    # deconv weights zero-padded to 128 rows:
    # wd4[32*jm + ci, (jm*4 + v)*128 + u*32 + co] = w_deconv[co, ci, u, v]
    wd_small = np.transpose(w_deconv, (1, 3, 2, 0)).reshape(OC, 4, 128)  # ci,v,uc
    wd_host = np.zeros((128, 2048), np.float32)
    for jm in range(4):
        for v in range(4):
            wd_host[32 * jm:32 * (jm + 1), (jm * 4 + v) * 128:
                    (jm * 4 + v + 1) * 128] = wd_small[:, v, :]
    wd_host = wd_host.astype(bf)

    eye_host = np.eye(B, dtype=np.float32).astype(bf)

    # pairwise sign matrix (496 pairs padded to 512) and incidence
    pairs = [(a, b) for a in range(B) for b in range(a + 1, B)]
    sgn_host = np.zeros((128, 512), np.float32)
    inc_host = np.zeros((128, 128), np.float32)
    for p, (a, b) in enumerate(pairs):
        for i in range(4):
            sgn_host[32 * i + a, p] = 1.0
            sgn_host[32 * i + b, p] = -1.0
        inc_host[p % 128, (p // 128) * B + a] = 1.0
        inc_host[p % 128, (p // 128) * B + b] = 1.0
    sgn_host = sgn_host.astype(bf)
    inc_host = inc_host.astype(bf)

    # T: (768, 8192, 16) f32; din=(k,p), d=(ch,i,j), i=(core,r), j=(jd,jm)
    # per-core dram layout [g=(r,jd)][p][k][jm][ch][f], x1024 -> fp8
    T8 = np.asarray(T, np.float32).reshape(KCH, 128, OC, 8, 2, 4, 4, F)
    T8 = np.clip(T8 * TS, -240, 240)
    # (k p ch c r jd jm f) -> (c, r, jd, p, k, jm, ch, f)
    T8 = np.ascontiguousarray(T8.transpose(3, 4, 5, 1, 0, 6, 2, 7)).astype(f8)
    T8 = T8.reshape(N_CORES, NG, 128, KCH * 2048)

    in_maps = []
    for c in range(N_CORES):
        in_maps.append({
            "zrow": np.zeros((1, 128), bf),
            "xc8": np.ascontiguousarray(
                xq[:, BC * c:BC * (c + 1)]).reshape(IN_FLT, BC * N * N),
            "tsh": T8[c],
            "wc": wc_host,
            "wd4": wd_host,
            "sgn4": sgn_host,
            "inc": inc_host,
            "eye": eye_host,
        })
    return in_maps


def _get_nc():
    if "nc" not in _CACHE:
        _CACHE["nc"] = _build_nc()
    return _CACHE["nc"]


def run(inputs, trace=False, trace_kwargs=None):
    """Run on hardware; returns (full_output, BassKernelResults)."""
    from concourse.bass_utils import run_bass_kernel_spmd
    nc = _get_nc()
    in_maps = _host_prep(inputs["x"], inputs["w_conv"], inputs["T"],
                         inputs["w_deconv"])
    res = run_bass_kernel_spmd(nc, in_maps, list(range(N_CORES)), trace=trace,
                               **(trace_kwargs or {}))
    x = np.asarray(inputs["x"], dtype=np.float32)
    full = np.empty((B, IN_FLT + OC, N, N), np.float32)
    full[:, :IN_FLT] = x
    for c in range(N_CORES):
        yv = np.asarray(res.results[c]["y"], dtype=np.float32).reshape(2, 4, OC, B, N)  # (r, u, co, b, w)
        for r in range(2):
            # dest dims are (b, co, u, w)
            full[:, IN_FLT:, 8 * c + 4 * r:8 * c + 4 * r + 4, :] = \
                yv[r].transpose(2, 1, 0, 3)
    return full, res


def kernel(**inputs) -> np.ndarray:
    out, _ = run(inputs, trace=False)
    return out
